# revision 16
# baseline (speedup 1.0000x reference)
"""AM-softmax loss kernel for 8 Trainium2 NeuronCores.

Problem: x [2048, 192] f32, W [100000, 192] f32, label [2048] int64.
    xn = x / ||x||_row
    wf = xn @ W.T                       # [N, C] logits (never materialized)
    tgt = wf[i, label[i]]
    numer = S*(tgt - M)
    Z = sum_c exp(S*wf[:, c]) - exp(S*tgt) + exp(numer)   # label column replaced
    loss = -mean(numer - log(Z))

Sharding: vocab/tensor parallel — W's class dim split 8 ways (12500+12 pad
classes per core). Each core computes its partial sum-exp per row with a
fused matmul->exp(accumulate) pipeline, the [128,16] partial-Z vectors are
AllReduced, and every core finishes the (identical) scalar loss.

Key design points:
  - fp8e4m3 + DoubleRow matmul: K=192 contracts in ONE pass (vs 2 for
    bf16) — the PE runs at 1.2 GHz here, so streamed columns are the
    scarce resource. W is pre-scaled by 16 on the host for fp8 mantissa
    range; the 1/16 is folded into the exp scale.
  - row normalization is applied to the stationary operand ON DEVICE
    (xt8 = bf16 xT * rnorm broadcast, cast to fp8), so the ACT exp uses
    an immediate scale — a per-partition scale AP costs ~0.5us/instr.
  - 1/sqrt via DVE bit-trick + 2 Newton steps: no Sqrt table load; the
    whole kernel uses one ACT table set (natural_log_exp: square/exp/ln).
  - ACT accum_out produces the row sums (no separate reduce pass).
  - label-column correction is analytic: Z += exp(S*tgt)*(exp(-S*M)-1),
    with tgt computed exactly in f32 from a host-gathered W[label].
    The 12 zero-pad classes per core contribute exp(0)=1 each; the exact
    -96 is folded into the same correction op.
"""

import os
import sys

for _p in ("/opt/trn_rl_repo", os.path.expanduser("~/.axon_site/_ro/trn_rl_repo")):
    if os.path.isdir(_p) and _p not in sys.path:
        sys.path.insert(0, _p)

import math
from contextlib import ExitStack

import ml_dtypes
import numpy as np

N, D, C = 2048, 192, 100000
S, MARG = 30.0, 0.2
NCORES = 8
CS = C // NCORES            # 12500 classes per core
CSP = 12512                 # padded shard width (16B-aligned pair stride)
NPAD = CSP - CS             # 12 zero classes per core -> Z += 12 each
NT = N // 128               # 16 row tiles
KH = D // 2                 # 96 partition rows in DoubleRow pair layout
CHUNK = 512                 # matmul free-dim (one PSUM bank)
GROUP = 2048                # ACT read width (4 PSUM banks)
WSCALE = 16.0               # host pre-scale on W for fp8 range
# per-tile class groups within a padded shard: 6 x 2048 + 224
GROUPS = [(g * GROUP, GROUP) for g in range(CSP // GROUP)]
if CSP % GROUP:
    GROUPS.append((CSP - CSP % GROUP, CSP % GROUP))
NG = len(GROUPS)

BF16 = ml_dtypes.bfloat16
FP8 = ml_dtypes.float8_e4m3

_cached = {}


def _build():
    import concourse.bass as bass
    import concourse.mybir as mybir
    import concourse.tile as tile
    from concourse import bacc

    f32 = mybir.dt.float32
    u32 = mybir.dt.uint32
    bf16 = mybir.dt.bfloat16
    fp8 = mybir.dt.float8e4
    AF = mybir.ActivationFunctionType
    ALU = mybir.AluOpType

    nc = bacc.Bacc(
        None, target_bir_lowering=False, num_devices=NCORES, name="am_fp8_final")

    xtb = nc.declare_dram_parameter("xtb", [KH, 2 * N], bf16, isOutput=False)
    xf = nc.declare_dram_parameter("xf", [N, D], bf16, isOutput=False)
    wt = nc.declare_dram_parameter("wt", [KH, 2 * CSP], fp8, isOutput=False)
    wl = nc.declare_dram_parameter("wl", [N, D], bf16, isOutput=False)
    out = nc.declare_dram_parameter("out", [1, 1], f32, isOutput=True)

    with tile.TileContext(nc) as tc, ExitStack() as ctx:
        persist = ctx.enter_context(tc.tile_pool(name="persist", bufs=1))
        scr = ctx.enter_context(tc.tile_pool(name="scr", bufs=3))
        psum_banks_per_slot = GROUP * 4 // 2048
        pp = ctx.enter_context(tc.tile_pool(
            name="pp", bufs=8 // psum_banks_per_slot, space="PSUM"))
        dram = ctx.enter_context(tc.tile_pool(name="dram", bufs=1, space="DRAM"))

        # ---- startup-critical inputs ----
        xf_sb = persist.tile([128, NT * D], bf16)
        for i in range(NT):
            nc.sync.dma_start(xf_sb[:, i * D:(i + 1) * D], xf[i * 128:(i + 1) * 128, :])
        xtb_sb = persist.tile([KH, 2 * N], bf16)
        nc.sync.dma_start(xtb_sb[:], xtb[:])
        wt_sb = []
        for g, (c0, w) in enumerate(GROUPS):
            wg = persist.tile([KH, 2 * w], fp8, name=f"wt_g{g}")
            for j in range(2):
                nc.sync.dma_start(
                    wg[:, j * w:(j + 1) * w], wt[:, j * CSP + c0:j * CSP + c0 + w])
            wt_sb.append(wg)

        # ---- row norms ||x||^2: tiles 0-7 on ACT, tiles 8-15 on DVE ----
        norm2 = persist.tile([128, NT], f32)
        NH = NT // 2
        for i in range(NH):
            sq = scr.tile([128, D], f32, tag="sq")
            nc.scalar.activation(
                sq[:], xf_sb[:, i * D:(i + 1) * D], AF.Square,
                accum_out=norm2[:, i:i + 1])
        sqv = persist.tile([128, NH * D], f32)
        nc.vector.tensor_mul(
            sqv[:], xf_sb[:, NH * D:NT * D], xf_sb[:, NH * D:NT * D])
        nc.vector.tensor_reduce(
            norm2[:, NH:NT], sqv[:].rearrange("p (t d) -> p t d", d=D),
            axis=mybir.AxisListType.X, op=ALU.add)

        # ---- rnorm = 1/sqrt(norm2): DVE bit-trick seed + 2 Newton steps ----
        magic = persist.tile([128, NT], u32)
        nc.vector.memset(magic[:], 0x5F3759DF)
        half_i = scr.tile([128, NT], u32, tag="bits")
        nc.vector.tensor_scalar(
            out=half_i[:], in0=norm2[:].bitcast(u32), scalar1=1, scalar2=None,
            op0=ALU.arith_shift_right)
        rn = persist.tile([128, NT], f32)
        nc.vector.tensor_tensor(
            out=rn[:].bitcast(u32), in0=magic[:], in1=half_i[:], op=ALU.subtract)
        t1 = persist.tile([128, NT], f32)
        for _ in range(2):
            nc.vector.tensor_mul(t1[:], rn[:], rn[:])
            nc.vector.tensor_mul(t1[:], t1[:], norm2[:])
            nc.vector.tensor_scalar(
                out=t1[:], in0=t1[:], scalar1=-0.5, scalar2=1.5,
                op0=ALU.mult, op1=ALU.add)
            nc.vector.tensor_mul(rn[:], rn[:], t1[:])

        # ---- broadcast rnorm along n and scale the stationary: xt8 ----
        ident = persist.tile([128, 128], f32)
        from concourse.masks import make_identity
        make_identity(nc, ident[:])
        ps_t = pp.tile([16, 128], f32, tag="ps")
        nc.tensor.transpose(ps_t[:], rn[:], ident[:])
        rn_t = scr.tile([16, 128], f32, tag="rnt")
        nc.vector.tensor_copy(rn_t[:], ps_t[:])
        rn_dram = dram.tile([16, 128], f32)
        nc.sync.dma_start(rn_dram[:], rn_t[:])
        rnb = persist.tile([KH, 2 * N], f32)
        rn_bcast_src = bass.AP(
            rn_dram.tensor, rn_dram.offset, [[0, KH], [0, 2], [1, N]])
        nc.sync.dma_start(rnb[:].rearrange("p (j n) -> p j n", j=2), rn_bcast_src)
        xt8 = persist.tile([KH, 2 * N], fp8)
        xt8_3 = xt8[:].rearrange("p (two n) -> p two n", two=2)
        xtb_3 = xtb_sb[:].rearrange("p (two n) -> p two n", two=2)
        rnb_3 = rnb[:].rearrange("p (two n) -> p two n", two=2)
        for i in range(NT):
            sl = (slice(None), slice(None), slice(i * 128, (i + 1) * 128))
            nc.vector.tensor_tensor(
                out=xt8_3[sl], in0=xtb_3[sl], in1=rnb_3[sl], op=ALU.mult)
        xt3 = xt8_3

        # ---- main loop: DoubleRow logits chunk -> exp -> row-accumulate ----
        NT1 = NT - 2            # tiles in the first (early) AllReduce
        zparts = persist.tile([128, NT * NG], f32)

        def do_tile(i):
            lhs = xt3[:, :, i * 128:(i + 1) * 128]
            for g, (c0, w) in enumerate(GROUPS):
                wg3 = wt_sb[g][:].rearrange("p (two n) -> p two n", two=2)
                ps = pp.tile([128, GROUP], f32, tag="ps", name=f"ps_{i}_{g}")
                nch = (w + CHUNK - 1) // CHUNK
                for c in range(nch):
                    cw = min(CHUNK, w - c * CHUNK)
                    nc.tensor.matmul(
                        ps[:, c * CHUNK:c * CHUNK + cw], lhs,
                        wg3[:, :, c * CHUNK:c * CHUNK + cw], start=True, stop=True,
                        perf_mode=mybir.MatmulPerfMode.DoubleRow)
                zcol = i * NG + g
                nc.scalar.activation(
                    ps[:, 0:w], ps[:, 0:w], AF.Exp, bias=0.0,
                    scale=S / WSCALE, accum_out=zparts[:, zcol:zcol + 1])

        for i in range(NT1):
            do_tile(i)

        # early AllReduce for tiles 0..NT1-1, hidden under the last 2 tiles
        Zl1 = persist.tile([128, NT1], f32)
        nc.vector.tensor_reduce(
            Zl1[:], zparts[:, 0:NT1 * NG].rearrange("p (t g) -> p t g", g=NG),
            axis=mybir.AxisListType.X, op=ALU.add)
        cc_in1 = dram.tile([128, NT1], f32)
        cc_out1 = dram.tile([128, NT1], f32, addr_space="Shared")
        nc.gpsimd.dma_start(cc_in1[:], Zl1[:])
        nc.gpsimd.collective_compute(
            "AllReduce", mybir.AluOpType.add,
            replica_groups=[list(range(NCORES))],
            ins=[cc_in1[:].opt()], outs=[cc_out1[:].opt()])

        for i in range(NT1, NT):
            do_tile(i)

        # ---- label dot (off the critical path; DVE is idle in main loop) ----
        wl_sb = persist.tile([128, NT * D], bf16)
        for i in range(NT):
            nc.sync.dma_start(wl_sb[:, i * D:(i + 1) * D], wl[i * 128:(i + 1) * 128, :])
        prod = persist.tile([128, NT * D], f32)
        nc.vector.tensor_mul(prod[:], xf_sb[:], wl_sb[:])
        rawdot = persist.tile([128, NT], f32)
        nc.vector.tensor_reduce(
            rawdot[:], prod[:].rearrange("p (t d) -> p t d", d=D),
            axis=mybir.AxisListType.X, op=ALU.add)
        tgt = persist.tile([128, NT], f32)          # xn . W[label]
        nc.vector.tensor_mul(tgt[:], rawdot[:], rn[:])

        # ---- second-phase partial Z (last 2 tiles) + small AllReduce ----
        Zl2 = persist.tile([128, NT - NT1], f32)
        nc.vector.tensor_reduce(
            Zl2[:],
            zparts[:, NT1 * NG:NT * NG].rearrange("p (t g) -> p t g", g=NG),
            axis=mybir.AxisListType.X, op=ALU.add)
        cc_in2 = dram.tile([128, NT - NT1], f32)
        cc_out2 = dram.tile([128, NT - NT1], f32, addr_space="Shared")
        nc.gpsimd.dma_start(cc_in2[:], Zl2[:])
        nc.gpsimd.collective_compute(
            "AllReduce", mybir.AluOpType.add,
            replica_groups=[list(range(NCORES))],
            ins=[cc_in2[:].opt()], outs=[cc_out2[:].opt()])
        Zg = persist.tile([128, NT], f32)
        nc.gpsimd.dma_start(Zg[:, 0:NT1], cc_out1[:])
        nc.gpsimd.dma_start(Zg[:, NT1:NT], cc_out2[:])

        # ---- label-column correction + loss ----
        te = persist.tile([128, NT], f32)
        nc.scalar.activation(te[:], tgt[:], AF.Exp, bias=0.0, scale=S)
        corr = persist.tile([128, NT], f32)
        nc.vector.tensor_scalar(
            out=corr[:], in0=te[:], scalar1=math.exp(-S * MARG) - 1.0,
            scalar2=-float(NPAD * NCORES), op0=ALU.mult, op1=ALU.add)
        Zc = persist.tile([128, NT], f32)
        nc.vector.tensor_add(Zc[:], Zg[:], corr[:])
        lnz = persist.tile([128, NT], f32)
        nc.scalar.activation(lnz[:], Zc[:], AF.Ln)
        numer = persist.tile([128, NT], f32)
        nc.vector.tensor_scalar(
            out=numer[:], in0=tgt[:], scalar1=S, scalar2=-S * MARG,
            op0=ALU.mult, op1=ALU.add)
        lneg = persist.tile([128, NT], f32)         # log Z - numer = -L
        nc.vector.tensor_sub(lneg[:], lnz[:], numer[:])
        lsum = persist.tile([128, 1], f32)
        nc.vector.tensor_reduce(
            lsum[:], lneg[:], axis=mybir.AxisListType.X, op=ALU.add)
        ones = persist.tile([128, 1], f32)
        nc.vector.memset(ones[:], 1.0)
        ps_fin = pp.tile([1, 1], f32, tag="ps")
        nc.tensor.matmul(ps_fin[:], lsum[:], ones[:], start=True, stop=True)
        final = persist.tile([1, 1], f32)
        nc.scalar.activation(final[:], ps_fin[:], AF.Copy, bias=0.0, scale=1.0 / N)
        nc.sync.dma_start(out[:], final[:])

    return nc


def _get_nc():
    if "nc" not in _cached:
        nc = _build()
        nc.compile()
        _cached["nc"] = nc
    return _cached["nc"]


def _pair_layout(m, dt):
    """[192, F] -> [96, 2*F] half-split pair layout: out[k, j*F+n] = m[96j+k, n]."""
    F = m.shape[1]
    return np.ascontiguousarray(
        m.reshape(2, KH, F).transpose(1, 0, 2).reshape(KH, 2 * F)).astype(dt)


def _prep_inputs(x, W, label):
    x = np.asarray(x, dtype=np.float32)
    W = np.asarray(W, dtype=np.float32)
    label = np.asarray(label).astype(np.int64)

    xtb = _pair_layout(np.ascontiguousarray(x.T), BF16)       # [96, 2*2048] bf16
    wl = np.ascontiguousarray(W[label])                       # [2048, 192] f32
    in_maps = []
    for r in range(NCORES):
        wtp = np.zeros((D, CSP), dtype=np.float32)
        wtp[:, :CS] = W[r * CS:(r + 1) * CS, :].T * WSCALE
        in_maps.append({"xtb": xtb, "xf": x.astype(BF16),
                        "wt": _pair_layout(wtp, FP8), "wl": wl.astype(BF16)})
    return in_maps


def kernel(x, W, label, trace=False):
    from concourse.bass_utils import run_bass_kernel_spmd

    nc = _get_nc()
    in_maps = _prep_inputs(x, W, label)
    res = run_bass_kernel_spmd(nc, in_maps, core_ids=list(range(NCORES)), trace=trace)
    _cached["last_result"] = res
    return np.asarray(res.results[0]["out"][0, 0], dtype=np.float32)
